# revision 37
# baseline (speedup 1.0000x reference)
"""Multi-head self-attention Trainium2 kernel (8 NeuronCores).

Problem: B=4, S=2048, D=1024, H=8 heads (HD=128).
  qkv = x @ qkv_w.T + qkv_b ; q,k,v = split(qkv)
  q = (q @ q_w.T + q_b)  (same k, v) -> [B,H,S,HD]
  scores = q k^T * HD^-0.5, masked softmax (attn_mask==1 -> -inf), o = attn @ v
  out = o @ out_w.T + out_b

Sharding: 8 cores = 4 batches x 2 head-groups (4 heads each).
Core c: batch b = c % 4, head-group g = c // 4.

Host-side algebraic folding: the qkv projection and per-stream q/k/v
projections are both linear, so they are composed into single effective
weights (W_eff = w @ qkv_w_slice), halving device matmul work. The
out-projection is row-parallel across head-groups; the two partial
outputs per batch are summed on host (the unshard step) with out_b.

Device flow per core (all matmuls bf16 with fp32 PSUM accumulation):
  qT_h[HD,S], kT_h[HD,S] = W x^T      (contraction over D on partitions)
  v[S, 4*HD]                          (natural layout)
  per head, per q-half (1024 q), software-pipelined 2 chunks deep:
    for kc in 16 k-chunks:
      sT = kT_h[:,kc]^T @ qT_h        [128 k, 1024 q]   (PE -> PSUM f32)
      p  = exp(SCALE * sT)            (ACT -> bf16 SBUF)
      pm = p * keepT[kc]              (DVE; keep = attn_mask.T == 0)
      oT += v[kc]^T-as-lhsT @ pm      -> oT[HD, q]      (PE, PSUM accum)
      dB += ones^T @ pm               broadcast denominator (PE, PSUM)
    oT_sb = oT * exp(-ln(dB))         softmax normalization (ACT+DVE -> bf16)
  out_partial[s,:] = sum_h oT_h[:,s_chunk]^T @ outwT_h   (+host bias/sum)
"""

import os
import sys
import types

sys.path.insert(0, "/opt/trn_rl_repo")

import numpy as np
import ml_dtypes

BF16 = ml_dtypes.bfloat16

B, S, D, H, HD = 4, 2048, 1024, 8, 128
HG = 2           # head groups
HPG = H // HG    # heads per group (4)
GD = HPG * HD    # dims per group (512)
SCALE = float(HD) ** -0.5
NKC = S // 128   # 16 k chunks
NSC = S // 128   # 16 s chunks
ND = D // 128    # 8 d chunks

_cached = {}


def _install_ntff_hook_shim():
    """The agent image's antenv lacks axon_hooks; shim it so trace works."""
    if "antenv.axon_hooks" in sys.modules:
        return
    try:
        import trn_agent_boot.trn_boot as _tb

        _hook = _tb._ntff_profile_via_ctypes("/opt/axon/libaxon_pjrt.so")
    except Exception:
        _hook = None
    _m = types.ModuleType("antenv.axon_hooks")
    _m.get_axon_ntff_profile_hook = lambda: _hook
    sys.modules["antenv.axon_hooks"] = _m


def _split_waits(nc, mybir, maxw=1):
    """Walrus in this image allows only one sync wait per instruction;
    hoist extra waits onto preceding NoOps on the same engine."""
    n_new = 0
    for fn in nc.m.functions:
        for bb in fn.blocks:
            newlist = []
            for inst in bb.instructions:
                si = inst.sync_info
                if si is not None and si.on_wait is not None and len(si.on_wait) > maxw:
                    waits = list(si.on_wait)
                    extra, keep = waits[:-maxw], waits[-maxw:]
                    while extra:
                        chunk, extra = extra[:maxw], extra[maxw:]
                        nop = mybir.InstNoOp(name=f"I-waitsplit-{nc.next_id()}")
                        nop.engine = inst.engine
                        nop.sync_info = mybir.SyncInfo(on_wait=chunk, on_update=[])
                        newlist.append(nop)
                        n_new += 1
                    si.on_wait = keep
                newlist.append(inst)
            bb.instructions = newlist
    return n_new


def _build_program(use_vbias=True, q_descale=1.0, k_descale=1.0):
    import concourse.bass as bass
    import concourse.mybir as mybir
    import concourse.tile as tile

    f32 = mybir.dt.float32
    bf16 = mybir.dt.bfloat16
    fp8 = mybir.dt.float8e4
    DR = mybir.MatmulPerfMode.DoubleRow
    Exp = mybir.ActivationFunctionType.Exp
    Ident = mybir.ActivationFunctionType.Identity
    Ln = mybir.ActivationFunctionType.Ln

    nc = bass.Bass()

    # DRAM parameters (per-core shards, pre-tiled on host).
    # q/k projections run in fp8(e4m3) DoubleRow mode: inputs are absmax-
    # scaled to +-240 on host with GLOBAL scales (the SPMD program, incl.
    # the descale immediates, is shared by all cores); the Ident copy's
    # scale undoes the quantization scaling. The d-chunk PAIR layout
    # [128, 2, .] feeds the 256-deep virtual contraction of DoubleRow.
    # v (and everything downstream of the softmax) stays bf16: fp8 there
    # would put ~4% noise directly on the output, while on q/k it only
    # perturbs softmax logits by ~1%.
    xT = nc.declare_dram_parameter("xT", [ND, 128, S], bf16, isOutput=False)
    xqT = nc.declare_dram_parameter("xqT", [ND // 2, 128, 2, S], fp8, isOutput=False)
    wqfT = nc.declare_dram_parameter("wqfT", [ND // 2, 128, 2, GD], fp8, isOutput=False)
    wkfT = nc.declare_dram_parameter("wkfT", [ND // 2, 128, 2, GD], fp8, isOutput=False)
    wvT = nc.declare_dram_parameter("wvT", [128, ND * GD], bf16, isOutput=False)
    bq = nc.declare_dram_parameter("bq", [128, HPG], f32, isOutput=False)
    bk = nc.declare_dram_parameter("bk", [128, HPG], f32, isOutput=False)
    bvrow = nc.declare_dram_parameter("bvrow", [1, GD], bf16, isOutput=False)
    outwT = nc.declare_dram_parameter("outwT", [128, HPG * D], bf16, isOutput=False)
    keepT = nc.declare_dram_parameter("keepT", [NKC, 128, S], bf16, isOutput=False)
    out = nc.declare_dram_parameter("out", [S, D], bf16, isOutput=True)

    with tile.TileContext(nc) as tc:
        import contextlib

        with contextlib.ExitStack() as ctx:
            # --- pools ---
            # xT and keepT share one 16-slot rotation of [128, S] bf16 tiles.
            p_big = ctx.enter_context(tc.tile_pool(name="big2k", bufs=16))
            p_pers = ctx.enter_context(tc.tile_pool(name="pers", bufs=1))
            p_pm = ctx.enter_context(tc.tile_pool(name="pm", bufs=12))
            p_sm = ctx.enter_context(tc.tile_pool(name="small", bufs=2))
            pp_big = ctx.enter_context(tc.tile_pool(name="ppbig", bufs=2, space="PSUM"))
            pp_sm = ctx.enter_context(tc.tile_pool(name="ppsm", bufs=4, space="PSUM"))

            # --- constants + small inputs ---
            ones128 = p_pers.tile([128, 128], bf16, tag="ones128", name="ones128")
            nc.vector.memset(ones128, 1.0)

            # --- loads. Everything bandwidth-significant goes on the sync
            # HW DGE queue in strict consumption order (the HBM pipe is the
            # resource). DMA aggregate bandwidth ramps with queue depth, so
            # keep transfers outstanding — but split the first fp8 x tile
            # into quarters so the first matmuls' critical bytes land fast.
            wqf_sb = [
                p_pers.tile([128, 2, GD], fp8, tag=f"wqf{t}", name=f"wqf{t}")
                for t in range(ND // 2)
            ]
            xq_tiles = [
                p_pers.tile([128, 2, S], fp8, tag=f"xq{t}", name=f"xq{t}")
                for t in range(ND // 2)
            ]
            # first matmul's bytes first: wqf0, then xq0 in quarters
            nc.sync.dma_start(out=wqf_sb[0], in_=wqfT[0])
            for qu in range(4):
                nc.sync.dma_start(
                    out=xq_tiles[0][:, :, qu * 512:(qu + 1) * 512],
                    in_=xqT[0][:, :, qu * 512:(qu + 1) * 512],
                )
            for t in range(1, ND // 2):
                nc.sync.dma_start(out=wqf_sb[t], in_=wqfT[t])
                nc.sync.dma_start(out=xq_tiles[t], in_=xqT[t])
            wkf_sb = [
                p_pers.tile([128, 2, GD], fp8, tag=f"wkf{t}", name=f"wkf{t}")
                for t in range(ND // 2)
            ]
            for t in range(ND // 2):
                nc.sync.dma_start(out=wkf_sb[t], in_=wkfT[t])
            # bf16 x (for the v projection, which stays bf16)
            xt_tiles = []
            for d in range(ND):
                t = p_big.tile([128, S], bf16, tag="big2k", name="big2k")
                nc.sync.dma_start(out=t, in_=xT[d])
                xt_tiles.append(t)
            w_merged = {}
            wv_sb = p_pers.tile([128, ND * GD], bf16, tag="wv", name="wv")
            nc.sync.dma_start(out=wv_sb, in_=wvT[:, :])
            w_merged["v"] = wv_sb

            bq_sb = p_pers.tile([128, HPG], f32, tag="bq", name="bq_sb")
            nc.scalar.dma_start(out=bq_sb, in_=bq[:, :])
            bk_sb = p_pers.tile([128, HPG], f32, tag="bk", name="bk_sb")
            nc.scalar.dma_start(out=bk_sb, in_=bk[:, :])
            bv_sb = None
            if use_vbias:
                bv_sb = p_pers.tile([1, GD], bf16, tag="bv", name="bv_sb")
                nc.scalar.dma_start(out=bv_sb, in_=bvrow[:, :])

            def w_sl(name, d):
                return w_merged[name][:, d * GD:(d + 1) * GD]

            def xT_sl(d, lo, hi):
                return xt_tiles[d][:, lo:hi]

            keep_tiles = [None] * NKC
            for kc in range(8):
                t = p_big.tile([128, S], bf16, tag="big2k", name="big2k")
                nc.sync.dma_start(out=t, in_=keepT[kc])
                keep_tiles[kc] = t

            def keep_sl(kc, lo, hi):
                return keep_tiles[kc][:, lo:hi]

            outw_merged = p_pers.tile([128, HPG * D], bf16, tag="outw", name="outw")
            nc.sync.dma_start(out=outw_merged, in_=outwT[:, :])
            outw_sb = [outw_merged[:, h * D:(h + 1) * D] for h in range(HPG)]

            # --- q/k projections, fp8 DoubleRow (4 pair-passes over the
            # 1024-deep contraction instead of 8 bf16 passes). The first 8
            # units run pair-major across 8 concurrent PSUM accumulators
            # (4 pp_sm tiles + halves of 2 pp_big tiles): while xq tiles
            # stream in, the PE always has 8 matmuls ready per pair,
            # staying busy enough to hold the HAM clock warm. Remaining
            # units run unit-major; the Ident copy applies the fp8 descale
            # and bias. ---
            NT = ND // 2
            qT_sb = [p_pers.tile([128, S], bf16, tag=f"qT{h}", name=f"qT{h}") for h in range(HPG)]
            kT_sb = [p_pers.tile([128, S], bf16, tag=f"kT{h}", name=f"kT{h}") for h in range(HPG)]

            units = []  # (stream, head, quarter)
            for wf, dst, bias, dsc in (
                (wqf_sb, qT_sb, bq_sb, q_descale),
                (wkf_sb, kT_sb, bk_sb, k_descale),
            ):
                for h in range(HPG):
                    for qu in range(4):
                        units.append((wf, dst, bias, dsc, h, qu))

            # PE warm-up: the HAM clock gate needs ~3.4us of sustained
            # activity to unthrottle. Burn the initial DMA wait on dummy
            # matmuls over the ones tile so the first real matmuls run at
            # 2.4GHz instead of 1.2.
            warm_ps = pp_sm.tile([128, 128], f32, tag="ppsm", name="ppsm")
            for _ in range(24):
                nc.tensor.matmul(warm_ps, lhsT=ones128, rhs=ones128,
                                 start=True, stop=True)

            NWIDE = 8  # first units, pair-major over 8 accumulators
            wide = units[:NWIDE]
            pss = [
                pp_sm.tile([128, 512], f32, tag="ppsm", name="ppsm")
                for _ in range(4)
            ]
            big01 = [
                pp_big.tile([128, 1024], f32, tag="ppbig", name="ppbig")
                for _ in range(2)
            ]
            pss += [big01[0][:, 0:512], big01[0][:, 512:1024],
                    big01[1][:, 0:512], big01[1][:, 512:1024]]
            for t in range(NT):
                for (wf, dst, bias, dsc, h, qu), ps in zip(wide, pss):
                    nc.tensor.matmul(
                        ps,
                        lhsT=wf[t][:, :, h * 128:(h + 1) * 128],
                        rhs=xq_tiles[t][:, :, qu * 512:(qu + 1) * 512],
                        start=(t == 0),
                        stop=(t == NT - 1),
                        perf_mode=DR,
                    )
            for (wf, dst, bias, dsc, h, qu), ps in zip(wide, pss):
                nc.scalar.activation(
                    out=dst[h][:, qu * 512:(qu + 1) * 512],
                    in_=ps,
                    func=Ident,
                    scale=dsc,
                    bias=bias[:, h:h + 1],
                )

            for wf, dst, bias, dsc, h, qu in units[NWIDE:]:
                ps = pp_sm.tile([128, 512], f32, tag="ppsm", name="ppsm")
                for t in range(NT):
                    nc.tensor.matmul(
                        ps,
                        lhsT=wf[t][:, :, h * 128:(h + 1) * 128],
                        rhs=xq_tiles[t][:, :, qu * 512:(qu + 1) * 512],
                        start=(t == 0),
                        stop=(t == NT - 1),
                        perf_mode=DR,
                    )
                nc.scalar.activation(
                    out=dst[h][:, qu * 512:(qu + 1) * 512],
                    in_=ps,
                    func=Ident,
                    scale=dsc,
                    bias=bias[:, h:h + 1],
                )

            v_sb = [p_pers.tile([128, GD], bf16, tag=f"v{sc}", name=f"v{sc}") for sc in range(NSC)]
            for sc in range(NSC):
                ps = pp_sm.tile([128, GD], f32, tag="ppsm", name="ppsm")
                for d in range(ND):
                    nc.tensor.matmul(
                        ps,
                        lhsT=xT_sl(d, sc * 128, (sc + 1) * 128),
                        rhs=w_sl("v", d),
                        start=(d == 0),
                        stop=(d == ND - 1) and not use_vbias,
                    )
                if use_vbias:
                    # bias via K=1 ones row
                    nc.tensor.matmul(
                        ps,
                        lhsT=ones128[0:1, :],
                        rhs=bv_sb,
                        start=False,
                        stop=True,
                    )
                nc.vector.tensor_copy(v_sb[sc], ps)

            # --- second half of keepT (reuses xT slots once proj done).
            # Issued on sync: these dma_starts BLOCK the issuing engine on
            # the slot-free semaphore, so they must not ride the scalar
            # queue where they would stall the attention exps. ---
            for kc in range(8, NKC):
                t = p_big.tile([128, S], bf16, tag="big2k", name="big2k")
                nc.sync.dma_start(out=t, in_=keepT[kc])
                keep_tiles[kc] = t

            # --- attention. A single software pipeline runs ACROSS unit
            # boundaries: the deferred work queue (oT/dB consumes plus each
            # unit's softmax normalization) drains inside the NEXT unit's
            # kc loop, so the PE never waits for a unit's ln/exp/mul chain
            # at a boundary, and the scores-PSUM slots recycle while the
            # trailing exps of the previous unit are still in flight. ---
            oT_sb = [p_pers.tile([128, S], bf16, tag=f"oT{h}", name=f"oT{h}") for h in range(HPG)]
            DELAY = 4
            work = []  # deferred emission closures, popped in order

            for h in range(HPG):
                for half in range(2):
                    q0 = half * 1024
                    o_ps = [pp_sm.tile([128, 512], f32, tag="ppsm", name="ppsm") for _ in range(2)]
                    d_ps = [pp_sm.tile([128, 512], f32, tag="ppsm", name="ppsm") for _ in range(2)]

                    def consume(kc, pm, o_ps=o_ps, d_ps=d_ps, h=h):
                        for qq in range(2):
                            nc.tensor.matmul(
                                o_ps[qq],
                                lhsT=v_sb[kc][:, h * 128:(h + 1) * 128],
                                rhs=pm[:, qq * 512:(qq + 1) * 512],
                                start=(kc == 0),
                                stop=(kc == NKC - 1),
                            )
                        for qq in range(2):
                            nc.tensor.matmul(
                                d_ps[qq],
                                lhsT=ones128,
                                rhs=pm[:, qq * 512:(qq + 1) * 512],
                                start=(kc == 0),
                                stop=(kc == NKC - 1),
                            )

                    def norm(o_ps=o_ps, d_ps=d_ps, h=h, q0=q0):
                        for qq in range(2):
                            # 1/d via exp(-ln(d)) on ACT (same table set as
                            # the score exps; DVE reciprocal is ~6cyc/elem)
                            lnd = p_sm.tile([128, 512], f32, tag="lnd", name="lnd")
                            nc.scalar.activation(out=lnd, in_=d_ps[qq], func=Ln)
                            rdb = p_sm.tile([128, 512], f32, tag="rdb", name="rdb")
                            nc.scalar.activation(out=rdb, in_=lnd, func=Exp, scale=-1.0)
                            nc.vector.tensor_mul(
                                oT_sb[h][:, q0 + qq * 512:q0 + (qq + 1) * 512],
                                o_ps[qq],
                                rdb,
                            )

                    for kc in range(NKC):
                        sT = pp_big.tile([128, 1024], f32, tag="ppbig", name="ppbig")
                        for nn in range(2):
                            nc.tensor.matmul(
                                sT[:, nn * 512:(nn + 1) * 512],
                                lhsT=kT_sb[h][:, kc * 128:(kc + 1) * 128],
                                rhs=qT_sb[h][:, q0 + nn * 512:q0 + (nn + 1) * 512],
                                start=True,
                                stop=True,
                            )
                        p = p_pm.tile([128, 1024], bf16, tag="pm", name="pm")
                        nc.scalar.activation(out=p, in_=sT, func=Exp, scale=SCALE)
                        pm = p_pm.tile([128, 1024], bf16, tag="pm", name="pm")
                        nc.vector.tensor_mul(
                            pm, p, keep_sl(kc, q0, q0 + 1024)
                        )
                        work.append(lambda kc=kc, pm=pm, c=consume: c(kc, pm))
                        while len(work) > DELAY:
                            work.pop(0)()
                    work.append(norm)
            while work:
                work.pop(0)()

            # --- output projection (partial; host adds the two groups + bias).
            # Partial output in bf16: halves the DVE copy cost and the
            # output DMA traffic; host accumulates in f32. ---
            for sc in range(NSC):
                ps = pp_big.tile([128, 1024], f32, tag="ppbig", name="ppbig")
                for h in range(HPG):
                    for nn in range(2):
                        nc.tensor.matmul(
                            ps[:, nn * 512:(nn + 1) * 512],
                            lhsT=oT_sb[h][:, sc * 128:(sc + 1) * 128],
                            rhs=outw_sb[h][:, nn * 512:(nn + 1) * 512],
                            start=(h == 0),
                            stop=(h == HPG - 1),
                        )
                osb = p_sm.tile([128, 1024], bf16, tag="osb", name="osb")
                nc.vector.tensor_copy(osb, ps)
                # alternate issue queues: the ~600ns per-dma_start sequencer
                # cost would otherwise serialize the final drain
                eng = nc.sync if sc % 2 == 0 else nc.scalar
                eng.dma_start(out=out[sc * 128:(sc + 1) * 128, :], in_=osb)

    _split_waits(nc, mybir, maxw=1)
    return nc


def _prep_core_inputs(x, attn_mask, qkv_w, qkv_b, q_w, q_b, k_w, k_b, v_w, v_b,
                      out_w):
    """Host-side: fold projections, shard, pre-transpose/tile, cast."""
    f = np.float32
    x = np.asarray(x, f)
    qkv_w = np.asarray(qkv_w, f)
    qkv_b = np.asarray(qkv_b, f)
    Ws = {}
    bs = {}
    for i, (w, b) in enumerate(((q_w, q_b), (k_w, k_b), (v_w, v_b))):
        w = np.asarray(w, f)
        b = np.asarray(b, f)
        sl = slice(i * D, (i + 1) * D)
        Ws[i] = w @ qkv_w[sl]              # [D, D] effective
        bs[i] = b + w @ qkv_b[sl]          # [D]
    out_wT = np.ascontiguousarray(np.asarray(out_w, f).T)  # [D(hd), D(model)]

    keepT = (np.asarray(attn_mask).T == 0).astype(BF16)    # [k, q]
    keepT_t = np.ascontiguousarray(keepT).reshape(NKC, 128, S)

    xT_all = []
    for b_i in range(B):
        xb = np.ascontiguousarray(x[b_i].T.astype(BF16))   # [D, S]
        xT_all.append(xb.reshape(ND, 128, S))

    def dmaj(w):
        # [D, GD] -> [128, ND*GD] with d-chunk-major free layout
        return np.ascontiguousarray(
            w.reshape(ND, 128, GD).transpose(1, 0, 2).reshape(128, ND * GD)
        )

    def pairs(a, ncol):
        # [D, ncol] f32 (already quant-scaled) -> fp8 [ND//2, 128, 2, ncol]
        q = np.clip(a, -240.0, 240.0).astype(ml_dtypes.float8_e4m3fn)
        return np.ascontiguousarray(
            q.reshape(ND // 2, 2, 128, ncol).transpose(0, 2, 1, 3)
        )

    # global fp8 quantization scales (must be identical across cores: the
    # descale immediate is baked into the shared SPMD program)
    sx = 240.0 / max(float(np.abs(x).max()), 1e-30)
    swq = 240.0 / max(float(np.abs(Ws[0]).max()), 1e-30)
    swk = 240.0 / max(float(np.abs(Ws[1]).max()), 1e-30)

    maps = []
    for c in range(8):
        b_i = c % B
        g = c // B
        sl = slice(g * GD, (g + 1) * GD)
        xb_f32 = x[b_i].T  # [D, S]
        m = {
            "xT": xT_all[b_i],
            "xqT": pairs(xb_f32 * sx, S),
            "wqfT": pairs(Ws[0][sl].T * swq, GD),
            "wkfT": pairs(Ws[1][sl].T * swk, GD),
            "wvT": dmaj(Ws[2][sl].T.astype(BF16)),
            "bq": np.ascontiguousarray(bs[0][sl].reshape(HPG, 128).T.astype(f)),
            "bk": np.ascontiguousarray(bs[1][sl].reshape(HPG, 128).T.astype(f)),
            "bvrow": bs[2][sl].astype(BF16).reshape(1, GD),
            "outwT": np.ascontiguousarray(
                out_wT[sl].astype(BF16).reshape(HPG, 128, D)
                .transpose(1, 0, 2).reshape(128, HPG * D)
            ),
            "keepT": keepT_t,
        }
        maps.append(m)
    return maps, (sx, swq, swk)


def kernel(x, attn_mask, qkv_w, qkv_b, q_w, q_b, k_w, k_b, v_w, v_b,
           out_w, out_b, _trace=False):
    _install_ntff_hook_shim()
    from concourse.bass_utils import run_bass_kernel_spmd

    in_maps, (sx, swq, swk) = _prep_core_inputs(
        x, attn_mask, qkv_w, qkv_b, q_w, q_b, k_w, k_b, v_w, v_b, out_w
    )
    use_vbias = bool(np.any(np.asarray(in_maps[0]["bvrow"], np.float32) != 0))
    q_descale = 1.0 / (sx * swq)
    k_descale = 1.0 / (sx * swk)
    key = ("nc", use_vbias, q_descale, k_descale)
    if key not in _cached:
        _cached[key] = _build_program(
            use_vbias=use_vbias, q_descale=q_descale, k_descale=k_descale
        )
    nc = _cached[key]
    core_ids = list(range(8))
    try:
        res = run_bass_kernel_spmd(nc, in_maps, core_ids, trace=_trace)
    except Exception:
        # transient NRT device wedge: reset the PJRT client best-effort,
        # then retry once
        try:
            import jax

            jax.clear_backends()
        except Exception:
            pass
        res = run_bass_kernel_spmd(nc, in_maps, core_ids, trace=_trace)
    _cached["last_result"] = res

    out_b = np.asarray(out_b, np.float32)
    full = np.empty((B, S, D), np.float32)
    for b_i in range(B):
        full[b_i] = (
            res.results[b_i]["out"].astype(np.float32)
            + res.results[b_i + B]["out"].astype(np.float32)
            + out_b
        )
    return full



# revision 38
# speedup vs baseline: 1.0141x; 1.0141x over previous
"""Multi-head self-attention Trainium2 kernel (8 NeuronCores).

Problem: B=4, S=2048, D=1024, H=8 heads (HD=128).
  qkv = x @ qkv_w.T + qkv_b ; q,k,v = split(qkv)
  q = (q @ q_w.T + q_b)  (same k, v) -> [B,H,S,HD]
  scores = q k^T * HD^-0.5, masked softmax (attn_mask==1 -> -inf), o = attn @ v
  out = o @ out_w.T + out_b

Sharding: 8 cores = 4 batches x 2 head-groups (4 heads each).
Core c: batch b = c % 4, head-group g = c // 4.

Host-side algebraic folding: the qkv projection and per-stream q/k/v
projections are both linear, so they are composed into single effective
weights (W_eff = w @ qkv_w_slice), halving device matmul work. The
out-projection is row-parallel across head-groups; the two partial
outputs per batch are summed on host (the unshard step) with out_b.

Device flow per core (all matmuls bf16 with fp32 PSUM accumulation):
  qT_h[HD,S], kT_h[HD,S] = W x^T      (contraction over D on partitions)
  v[S, 4*HD]                          (natural layout)
  per head, per q-half (1024 q), software-pipelined 2 chunks deep:
    for kc in 16 k-chunks:
      sT = kT_h[:,kc]^T @ qT_h        [128 k, 1024 q]   (PE -> PSUM f32)
      p  = exp(SCALE * sT)            (ACT -> bf16 SBUF)
      pm = p * keepT[kc]              (DVE; keep = attn_mask.T == 0)
      oT += v[kc]^T-as-lhsT @ pm      -> oT[HD, q]      (PE, PSUM accum)
      dB += ones^T @ pm               broadcast denominator (PE, PSUM)
    oT_sb = oT * exp(-ln(dB))         softmax normalization (ACT+DVE -> bf16)
  out_partial[s,:] = sum_h oT_h[:,s_chunk]^T @ outwT_h   (+host bias/sum)
"""

import os
import sys
import types

sys.path.insert(0, "/opt/trn_rl_repo")

import numpy as np
import ml_dtypes

BF16 = ml_dtypes.bfloat16

B, S, D, H, HD = 4, 2048, 1024, 8, 128
HG = 2           # head groups
HPG = H // HG    # heads per group (4)
GD = HPG * HD    # dims per group (512)
SCALE = float(HD) ** -0.5
NKC = S // 128   # 16 k chunks
NSC = S // 128   # 16 s chunks
ND = D // 128    # 8 d chunks

_cached = {}


def _install_ntff_hook_shim():
    """The agent image's antenv lacks axon_hooks; shim it so trace works."""
    if "antenv.axon_hooks" in sys.modules:
        return
    try:
        import trn_agent_boot.trn_boot as _tb

        _hook = _tb._ntff_profile_via_ctypes("/opt/axon/libaxon_pjrt.so")
    except Exception:
        _hook = None
    _m = types.ModuleType("antenv.axon_hooks")
    _m.get_axon_ntff_profile_hook = lambda: _hook
    sys.modules["antenv.axon_hooks"] = _m


def _split_waits(nc, mybir, maxw=1):
    """Walrus in this image allows only one sync wait per instruction;
    hoist extra waits onto preceding NoOps on the same engine."""
    n_new = 0
    for fn in nc.m.functions:
        for bb in fn.blocks:
            newlist = []
            for inst in bb.instructions:
                si = inst.sync_info
                if si is not None and si.on_wait is not None and len(si.on_wait) > maxw:
                    waits = list(si.on_wait)
                    extra, keep = waits[:-maxw], waits[-maxw:]
                    while extra:
                        chunk, extra = extra[:maxw], extra[maxw:]
                        nop = mybir.InstNoOp(name=f"I-waitsplit-{nc.next_id()}")
                        nop.engine = inst.engine
                        nop.sync_info = mybir.SyncInfo(on_wait=chunk, on_update=[])
                        newlist.append(nop)
                        n_new += 1
                    si.on_wait = keep
                newlist.append(inst)
            bb.instructions = newlist
    return n_new


def _build_program(use_vbias=True, q_descale=1.0, k_descale=1.0):
    import concourse.bass as bass
    import concourse.mybir as mybir
    import concourse.tile as tile

    f32 = mybir.dt.float32
    bf16 = mybir.dt.bfloat16
    fp8 = mybir.dt.float8e4
    DR = mybir.MatmulPerfMode.DoubleRow
    Exp = mybir.ActivationFunctionType.Exp
    Ident = mybir.ActivationFunctionType.Identity
    Ln = mybir.ActivationFunctionType.Ln

    nc = bass.Bass()

    # DRAM parameters (per-core shards, pre-tiled on host).
    # q/k projections run in fp8(e4m3) DoubleRow mode: inputs are absmax-
    # scaled to +-240 on host with GLOBAL scales (the SPMD program, incl.
    # the descale immediates, is shared by all cores); the Ident copy's
    # scale undoes the quantization scaling. The d-chunk PAIR layout
    # [128, 2, .] feeds the 256-deep virtual contraction of DoubleRow.
    # v (and everything downstream of the softmax) stays bf16: fp8 there
    # would put ~4% noise directly on the output, while on q/k it only
    # perturbs softmax logits by ~1%.
    xT = nc.declare_dram_parameter("xT", [ND, 128, S], bf16, isOutput=False)
    xqT = nc.declare_dram_parameter("xqT", [ND // 2, 128, 2, S], fp8, isOutput=False)
    wqfT = nc.declare_dram_parameter("wqfT", [ND // 2, 128, 2, GD], fp8, isOutput=False)
    wkfT = nc.declare_dram_parameter("wkfT", [ND // 2, 128, 2, GD], fp8, isOutput=False)
    wvT = nc.declare_dram_parameter("wvT", [128, ND * GD], bf16, isOutput=False)
    bq = nc.declare_dram_parameter("bq", [128, HPG], f32, isOutput=False)
    bk = nc.declare_dram_parameter("bk", [128, HPG], f32, isOutput=False)
    bvrow = nc.declare_dram_parameter("bvrow", [1, GD], bf16, isOutput=False)
    outwT = nc.declare_dram_parameter("outwT", [128, HPG * D], bf16, isOutput=False)
    keepT = nc.declare_dram_parameter("keepT", [NKC, 128, S], bf16, isOutput=False)
    out = nc.declare_dram_parameter("out", [S, D], bf16, isOutput=True)

    with tile.TileContext(nc) as tc:
        import contextlib

        with contextlib.ExitStack() as ctx:
            # --- pools ---
            # xT and keepT share one 16-slot rotation of [128, S] bf16 tiles.
            p_big = ctx.enter_context(tc.tile_pool(name="big2k", bufs=16))
            p_pers = ctx.enter_context(tc.tile_pool(name="pers", bufs=1))
            p_pm = ctx.enter_context(tc.tile_pool(name="pm", bufs=12))
            p_sm = ctx.enter_context(tc.tile_pool(name="small", bufs=2))
            pp_big = ctx.enter_context(tc.tile_pool(name="ppbig", bufs=2, space="PSUM"))
            pp_sm = ctx.enter_context(tc.tile_pool(name="ppsm", bufs=4, space="PSUM"))

            # --- constants + small inputs ---
            ones128 = p_pers.tile([128, 128], bf16, tag="ones128", name="ones128")
            nc.vector.memset(ones128, 1.0)

            # --- loads. Everything bandwidth-significant goes on the sync
            # HW DGE queue in strict consumption order (the HBM pipe is the
            # resource). DMA aggregate bandwidth ramps with queue depth, so
            # keep transfers outstanding — but split the first fp8 x tile
            # into quarters so the first matmuls' critical bytes land fast.
            wqf_sb = [
                p_pers.tile([128, 2, GD], fp8, tag=f"wqf{t}", name=f"wqf{t}")
                for t in range(ND // 2)
            ]
            xq_tiles = [
                p_pers.tile([128, 2, S], fp8, tag=f"xq{t}", name=f"xq{t}")
                for t in range(ND // 2)
            ]
            # first matmul's bytes first: wqf0, then xq0 in quarters
            nc.sync.dma_start(out=wqf_sb[0], in_=wqfT[0])
            for qu in range(4):
                nc.sync.dma_start(
                    out=xq_tiles[0][:, :, qu * 512:(qu + 1) * 512],
                    in_=xqT[0][:, :, qu * 512:(qu + 1) * 512],
                )
            for t in range(1, ND // 2):
                nc.sync.dma_start(out=wqf_sb[t], in_=wqfT[t])
                nc.sync.dma_start(out=xq_tiles[t], in_=xqT[t])
            wkf_sb = [
                p_pers.tile([128, 2, GD], fp8, tag=f"wkf{t}", name=f"wkf{t}")
                for t in range(ND // 2)
            ]
            for t in range(ND // 2):
                nc.sync.dma_start(out=wkf_sb[t], in_=wkfT[t])
            # bf16 x (for the v projection, which stays bf16)
            xt_tiles = []
            for d in range(ND):
                t = p_big.tile([128, S], bf16, tag="big2k", name="big2k")
                nc.sync.dma_start(out=t, in_=xT[d])
                xt_tiles.append(t)
            w_merged = {}
            wv_sb = p_pers.tile([128, ND * GD], bf16, tag="wv", name="wv")
            nc.sync.dma_start(out=wv_sb, in_=wvT[:, :])
            w_merged["v"] = wv_sb

            bq_sb = p_pers.tile([128, HPG], f32, tag="bq", name="bq_sb")
            nc.scalar.dma_start(out=bq_sb, in_=bq[:, :])
            bk_sb = p_pers.tile([128, HPG], f32, tag="bk", name="bk_sb")
            nc.scalar.dma_start(out=bk_sb, in_=bk[:, :])
            bv_sb = None
            if use_vbias:
                bv_sb = p_pers.tile([1, GD], bf16, tag="bv", name="bv_sb")
                nc.scalar.dma_start(out=bv_sb, in_=bvrow[:, :])

            def w_sl(name, d):
                return w_merged[name][:, d * GD:(d + 1) * GD]

            def xT_sl(d, lo, hi):
                return xt_tiles[d][:, lo:hi]

            keep_tiles = [None] * NKC
            for kc in range(8):
                t = p_big.tile([128, S], bf16, tag="big2k", name="big2k")
                nc.sync.dma_start(out=t, in_=keepT[kc])
                keep_tiles[kc] = t

            def keep_sl(kc, lo, hi):
                return keep_tiles[kc][:, lo:hi]

            outw_merged = p_pers.tile([128, HPG * D], bf16, tag="outw", name="outw")
            nc.sync.dma_start(out=outw_merged, in_=outwT[:, :])
            outw_sb = [outw_merged[:, h * D:(h + 1) * D] for h in range(HPG)]

            # --- q/k projections, fp8 DoubleRow (4 pair-passes over the
            # 1024-deep contraction instead of 8 bf16 passes). The first 8
            # units run pair-major across 8 concurrent PSUM accumulators
            # (4 pp_sm tiles + halves of 2 pp_big tiles): while xq tiles
            # stream in, the PE always has 8 matmuls ready per pair,
            # staying busy enough to hold the HAM clock warm. Remaining
            # units run unit-major; the Ident copy applies the fp8 descale
            # and bias. ---
            NT = ND // 2
            qT_sb = [p_pers.tile([128, S], bf16, tag=f"qT{h}", name=f"qT{h}") for h in range(HPG)]
            kT_sb = [p_pers.tile([128, S], bf16, tag=f"kT{h}", name=f"kT{h}") for h in range(HPG)]

            units = []  # (stream, head, quarter)
            for wf, dst, bias, dsc in (
                (wqf_sb, qT_sb, bq_sb, q_descale),
                (wkf_sb, kT_sb, bk_sb, k_descale),
            ):
                for h in range(HPG):
                    for qu in range(4):
                        units.append((wf, dst, bias, dsc, h, qu))

            # PE warm-up: the HAM clock gate needs ~3.4us of sustained
            # activity to unthrottle. Burn the initial DMA wait on dummy
            # matmuls over the ones tile so the first real matmuls run at
            # 2.4GHz instead of 1.2.
            warm_ps = pp_sm.tile([128, 128], f32, tag="ppsm", name="ppsm")
            for _ in range(24):
                nc.tensor.matmul(warm_ps, lhsT=ones128, rhs=ones128,
                                 start=True, stop=True)

            NWIDE = 8  # first units, pair-major over 8 accumulators
            wide = units[:NWIDE]
            pss = [
                pp_sm.tile([128, 512], f32, tag="ppsm", name="ppsm")
                for _ in range(4)
            ]
            big01 = [
                pp_big.tile([128, 1024], f32, tag="ppbig", name="ppbig")
                for _ in range(2)
            ]
            pss += [big01[0][:, 0:512], big01[0][:, 512:1024],
                    big01[1][:, 0:512], big01[1][:, 512:1024]]
            for t in range(NT):
                for (wf, dst, bias, dsc, h, qu), ps in zip(wide, pss):
                    nc.tensor.matmul(
                        ps,
                        lhsT=wf[t][:, :, h * 128:(h + 1) * 128],
                        rhs=xq_tiles[t][:, :, qu * 512:(qu + 1) * 512],
                        start=(t == 0),
                        stop=(t == NT - 1),
                        perf_mode=DR,
                    )
            for (wf, dst, bias, dsc, h, qu), ps in zip(wide, pss):
                nc.scalar.activation(
                    out=dst[h][:, qu * 512:(qu + 1) * 512],
                    in_=ps,
                    func=Ident,
                    scale=dsc,
                    bias=bias[:, h:h + 1],
                )

            for wf, dst, bias, dsc, h, qu in units[NWIDE:]:
                ps = pp_sm.tile([128, 512], f32, tag="ppsm", name="ppsm")
                for t in range(NT):
                    nc.tensor.matmul(
                        ps,
                        lhsT=wf[t][:, :, h * 128:(h + 1) * 128],
                        rhs=xq_tiles[t][:, :, qu * 512:(qu + 1) * 512],
                        start=(t == 0),
                        stop=(t == NT - 1),
                        perf_mode=DR,
                    )
                nc.scalar.activation(
                    out=dst[h][:, qu * 512:(qu + 1) * 512],
                    in_=ps,
                    func=Ident,
                    scale=dsc,
                    bias=bias[:, h:h + 1],
                )

            v_sb = [p_pers.tile([128, GD], bf16, tag=f"v{sc}", name=f"v{sc}") for sc in range(NSC)]
            for sc in range(NSC):
                ps = pp_sm.tile([128, GD], f32, tag="ppsm", name="ppsm")
                for d in range(ND):
                    nc.tensor.matmul(
                        ps,
                        lhsT=xT_sl(d, sc * 128, (sc + 1) * 128),
                        rhs=w_sl("v", d),
                        start=(d == 0),
                        stop=(d == ND - 1) and not use_vbias,
                    )
                if use_vbias:
                    # bias via K=1 ones row
                    nc.tensor.matmul(
                        ps,
                        lhsT=ones128[0:1, :],
                        rhs=bv_sb,
                        start=False,
                        stop=True,
                    )
                nc.vector.tensor_copy(v_sb[sc], ps)

            # --- second half of keepT (reuses xT slots once proj done).
            # Issued on sync: these dma_starts BLOCK the issuing engine on
            # the slot-free semaphore, so they must not ride the scalar
            # queue where they would stall the attention exps. ---
            for kc in range(8, NKC):
                t = p_big.tile([128, S], bf16, tag="big2k", name="big2k")
                nc.sync.dma_start(out=t, in_=keepT[kc])
                keep_tiles[kc] = t

            # --- attention. A single software pipeline runs ACROSS unit
            # boundaries: the deferred work queue (oT/dB consumes plus each
            # unit's softmax normalization) drains inside the NEXT unit's
            # kc loop, so the PE never waits for a unit's ln/exp/mul chain
            # at a boundary, and the scores-PSUM slots recycle while the
            # trailing exps of the previous unit are still in flight. ---
            oT_sb = [p_pers.tile([128, S], bf16, tag=f"oT{h}", name=f"oT{h}") for h in range(HPG)]
            DELAY = 4
            work = []  # deferred emission closures, popped in order

            for h in range(HPG):
                for half in range(2):
                    q0 = half * 1024
                    o_ps = [pp_sm.tile([128, 512], f32, tag="ppsm", name="ppsm") for _ in range(2)]
                    d_ps = [pp_sm.tile([128, 512], f32, tag="ppsm", name="ppsm") for _ in range(2)]

                    def consume(kc, pm, o_ps=o_ps, d_ps=d_ps, h=h):
                        for qq in range(2):
                            nc.tensor.matmul(
                                o_ps[qq],
                                lhsT=v_sb[kc][:, h * 128:(h + 1) * 128],
                                rhs=pm[:, qq * 512:(qq + 1) * 512],
                                start=(kc == 0),
                                stop=(kc == NKC - 1),
                            )
                        for qq in range(2):
                            nc.tensor.matmul(
                                d_ps[qq],
                                lhsT=ones128,
                                rhs=pm[:, qq * 512:(qq + 1) * 512],
                                start=(kc == 0),
                                stop=(kc == NKC - 1),
                            )

                    def norm(o_ps=o_ps, d_ps=d_ps, h=h, q0=q0):
                        for qq in range(2):
                            # 1/d via exp(-ln(d)) on ACT (same table set as
                            # the score exps; DVE reciprocal is ~6cyc/elem)
                            lnd = p_sm.tile([128, 512], f32, tag="lnd", name="lnd")
                            nc.scalar.activation(out=lnd, in_=d_ps[qq], func=Ln)
                            rdb = p_sm.tile([128, 512], f32, tag="rdb", name="rdb")
                            nc.scalar.activation(out=rdb, in_=lnd, func=Exp, scale=-1.0)
                            nc.vector.tensor_mul(
                                oT_sb[h][:, q0 + qq * 512:q0 + (qq + 1) * 512],
                                o_ps[qq],
                                rdb,
                            )

                    for kc in range(NKC):
                        sT = pp_big.tile([128, 1024], f32, tag="ppbig", name="ppbig")
                        for nn in range(2):
                            nc.tensor.matmul(
                                sT[:, nn * 512:(nn + 1) * 512],
                                lhsT=kT_sb[h][:, kc * 128:(kc + 1) * 128],
                                rhs=qT_sb[h][:, q0 + nn * 512:q0 + (nn + 1) * 512],
                                start=True,
                                stop=True,
                            )
                        p = p_pm.tile([128, 1024], bf16, tag="pm", name="pm")
                        nc.scalar.activation(out=p, in_=sT, func=Exp, scale=SCALE)
                        pm = p_pm.tile([128, 1024], bf16, tag="pm", name="pm")
                        nc.vector.tensor_mul(
                            pm, p, keep_sl(kc, q0, q0 + 1024)
                        )
                        work.append(lambda kc=kc, pm=pm, c=consume: c(kc, pm))
                        while len(work) > DELAY:
                            work.pop(0)()
                    work.append(norm)
            while work:
                work.pop(0)()

            # --- output projection (partial; host adds the two groups + bias).
            # Partial output in bf16: halves the DVE copy cost and the
            # output DMA traffic; host accumulates in f32. ---
            for sc in range(NSC):
                ps = pp_big.tile([128, 1024], f32, tag="ppbig", name="ppbig")
                for h in range(HPG):
                    for nn in range(2):
                        nc.tensor.matmul(
                            ps[:, nn * 512:(nn + 1) * 512],
                            lhsT=oT_sb[h][:, sc * 128:(sc + 1) * 128],
                            rhs=outw_sb[h][:, nn * 512:(nn + 1) * 512],
                            start=(h == 0),
                            stop=(h == HPG - 1),
                        )
                osb = p_sm.tile([128, 1024], bf16, tag="osb", name="osb")
                # copy on ACT (idle once the exps are done); DVE still owns
                # the last unit's mask/normalization work at this point
                nc.scalar.activation(out=osb, in_=ps, func=Ident)
                # out DMAs issue from sync: scalar must stay unblocked for
                # the Ident copies
                nc.sync.dma_start(out=out[sc * 128:(sc + 1) * 128, :], in_=osb)

    _split_waits(nc, mybir, maxw=1)
    return nc


def _prep_core_inputs(x, attn_mask, qkv_w, qkv_b, q_w, q_b, k_w, k_b, v_w, v_b,
                      out_w):
    """Host-side: fold projections, shard, pre-transpose/tile, cast."""
    f = np.float32
    x = np.asarray(x, f)
    qkv_w = np.asarray(qkv_w, f)
    qkv_b = np.asarray(qkv_b, f)
    Ws = {}
    bs = {}
    for i, (w, b) in enumerate(((q_w, q_b), (k_w, k_b), (v_w, v_b))):
        w = np.asarray(w, f)
        b = np.asarray(b, f)
        sl = slice(i * D, (i + 1) * D)
        Ws[i] = w @ qkv_w[sl]              # [D, D] effective
        bs[i] = b + w @ qkv_b[sl]          # [D]
    out_wT = np.ascontiguousarray(np.asarray(out_w, f).T)  # [D(hd), D(model)]

    keepT = (np.asarray(attn_mask).T == 0).astype(BF16)    # [k, q]
    keepT_t = np.ascontiguousarray(keepT).reshape(NKC, 128, S)

    xT_all = []
    for b_i in range(B):
        xb = np.ascontiguousarray(x[b_i].T.astype(BF16))   # [D, S]
        xT_all.append(xb.reshape(ND, 128, S))

    def dmaj(w):
        # [D, GD] -> [128, ND*GD] with d-chunk-major free layout
        return np.ascontiguousarray(
            w.reshape(ND, 128, GD).transpose(1, 0, 2).reshape(128, ND * GD)
        )

    def pairs(a, ncol):
        # [D, ncol] f32 (already quant-scaled) -> fp8 [ND//2, 128, 2, ncol]
        q = np.clip(a, -240.0, 240.0).astype(ml_dtypes.float8_e4m3fn)
        return np.ascontiguousarray(
            q.reshape(ND // 2, 2, 128, ncol).transpose(0, 2, 1, 3)
        )

    # global fp8 quantization scales (must be identical across cores: the
    # descale immediate is baked into the shared SPMD program)
    sx = 240.0 / max(float(np.abs(x).max()), 1e-30)
    swq = 240.0 / max(float(np.abs(Ws[0]).max()), 1e-30)
    swk = 240.0 / max(float(np.abs(Ws[1]).max()), 1e-30)

    maps = []
    for c in range(8):
        b_i = c % B
        g = c // B
        sl = slice(g * GD, (g + 1) * GD)
        xb_f32 = x[b_i].T  # [D, S]
        m = {
            "xT": xT_all[b_i],
            "xqT": pairs(xb_f32 * sx, S),
            "wqfT": pairs(Ws[0][sl].T * swq, GD),
            "wkfT": pairs(Ws[1][sl].T * swk, GD),
            "wvT": dmaj(Ws[2][sl].T.astype(BF16)),
            "bq": np.ascontiguousarray(bs[0][sl].reshape(HPG, 128).T.astype(f)),
            "bk": np.ascontiguousarray(bs[1][sl].reshape(HPG, 128).T.astype(f)),
            "bvrow": bs[2][sl].astype(BF16).reshape(1, GD),
            "outwT": np.ascontiguousarray(
                out_wT[sl].astype(BF16).reshape(HPG, 128, D)
                .transpose(1, 0, 2).reshape(128, HPG * D)
            ),
            "keepT": keepT_t,
        }
        maps.append(m)
    return maps, (sx, swq, swk)


def kernel(x, attn_mask, qkv_w, qkv_b, q_w, q_b, k_w, k_b, v_w, v_b,
           out_w, out_b, _trace=False):
    _install_ntff_hook_shim()
    from concourse.bass_utils import run_bass_kernel_spmd

    in_maps, (sx, swq, swk) = _prep_core_inputs(
        x, attn_mask, qkv_w, qkv_b, q_w, q_b, k_w, k_b, v_w, v_b, out_w
    )
    use_vbias = bool(np.any(np.asarray(in_maps[0]["bvrow"], np.float32) != 0))
    q_descale = 1.0 / (sx * swq)
    k_descale = 1.0 / (sx * swk)
    key = ("nc", use_vbias, q_descale, k_descale)
    if key not in _cached:
        _cached[key] = _build_program(
            use_vbias=use_vbias, q_descale=q_descale, k_descale=k_descale
        )
    nc = _cached[key]
    core_ids = list(range(8))
    try:
        res = run_bass_kernel_spmd(nc, in_maps, core_ids, trace=_trace)
    except Exception:
        # transient NRT device wedge: reset the PJRT client best-effort,
        # then retry once
        try:
            import jax

            jax.clear_backends()
        except Exception:
            pass
        res = run_bass_kernel_spmd(nc, in_maps, core_ids, trace=_trace)
    _cached["last_result"] = res

    out_b = np.asarray(out_b, np.float32)
    full = np.empty((B, S, D), np.float32)
    for b_i in range(B):
        full[b_i] = (
            res.results[b_i]["out"].astype(np.float32)
            + res.results[b_i + B]["out"].astype(np.float32)
            + out_b
        )
    return full



# revision 39
# speedup vs baseline: 1.0196x; 1.0054x over previous
"""Multi-head self-attention Trainium2 kernel (8 NeuronCores).

Problem: B=4, S=2048, D=1024, H=8 heads (HD=128).
  qkv = x @ qkv_w.T + qkv_b ; q,k,v = split(qkv)
  q = (q @ q_w.T + q_b)  (same k, v) -> [B,H,S,HD]
  scores = q k^T * HD^-0.5, masked softmax (attn_mask==1 -> -inf), o = attn @ v
  out = o @ out_w.T + out_b

Sharding: 8 cores = 4 batches x 2 head-groups (4 heads each).
Core c: batch b = c % 4, head-group g = c // 4.

Host-side algebraic folding: the qkv projection and per-stream q/k/v
projections are both linear, so they are composed into single effective
weights (W_eff = w @ qkv_w_slice), halving device matmul work. The
out-projection is row-parallel across head-groups; the two partial
outputs per batch are summed on host (the unshard step) with out_b.

Device flow per core (all matmuls bf16 with fp32 PSUM accumulation):
  qT_h[HD,S], kT_h[HD,S] = W x^T      (contraction over D on partitions)
  v[S, 4*HD]                          (natural layout)
  per head, per q-half (1024 q), software-pipelined 2 chunks deep:
    for kc in 16 k-chunks:
      sT = kT_h[:,kc]^T @ qT_h        [128 k, 1024 q]   (PE -> PSUM f32)
      p  = exp(SCALE * sT)            (ACT -> bf16 SBUF)
      pm = p * keepT[kc]              (DVE; keep = attn_mask.T == 0)
      oT += v[kc]^T-as-lhsT @ pm      -> oT[HD, q]      (PE, PSUM accum)
      dB += ones^T @ pm               broadcast denominator (PE, PSUM)
    oT_sb = oT * exp(-ln(dB))         softmax normalization (ACT+DVE -> bf16)
  out_partial[s,:] = sum_h oT_h[:,s_chunk]^T @ outwT_h   (+host bias/sum)
"""

import os
import sys
import types

sys.path.insert(0, "/opt/trn_rl_repo")

import numpy as np
import ml_dtypes

BF16 = ml_dtypes.bfloat16

B, S, D, H, HD = 4, 2048, 1024, 8, 128
HG = 2           # head groups
HPG = H // HG    # heads per group (4)
GD = HPG * HD    # dims per group (512)
SCALE = float(HD) ** -0.5
NKC = S // 128   # 16 k chunks
NSC = S // 128   # 16 s chunks
ND = D // 128    # 8 d chunks

_cached = {}
WARMUP_MMS = 0


def _install_ntff_hook_shim():
    """The agent image's antenv lacks axon_hooks; shim it so trace works."""
    if "antenv.axon_hooks" in sys.modules:
        return
    try:
        import trn_agent_boot.trn_boot as _tb

        _hook = _tb._ntff_profile_via_ctypes("/opt/axon/libaxon_pjrt.so")
    except Exception:
        _hook = None
    _m = types.ModuleType("antenv.axon_hooks")
    _m.get_axon_ntff_profile_hook = lambda: _hook
    sys.modules["antenv.axon_hooks"] = _m


def _split_waits(nc, mybir, maxw=1):
    """Walrus in this image allows only one sync wait per instruction;
    hoist extra waits onto preceding NoOps on the same engine."""
    n_new = 0
    for fn in nc.m.functions:
        for bb in fn.blocks:
            newlist = []
            for inst in bb.instructions:
                si = inst.sync_info
                if si is not None and si.on_wait is not None and len(si.on_wait) > maxw:
                    waits = list(si.on_wait)
                    extra, keep = waits[:-maxw], waits[-maxw:]
                    while extra:
                        chunk, extra = extra[:maxw], extra[maxw:]
                        nop = mybir.InstNoOp(name=f"I-waitsplit-{nc.next_id()}")
                        nop.engine = inst.engine
                        nop.sync_info = mybir.SyncInfo(on_wait=chunk, on_update=[])
                        newlist.append(nop)
                        n_new += 1
                    si.on_wait = keep
                newlist.append(inst)
            bb.instructions = newlist
    return n_new


def _build_program(use_vbias=True, q_descale=1.0, k_descale=1.0):
    import concourse.bass as bass
    import concourse.mybir as mybir
    import concourse.tile as tile

    f32 = mybir.dt.float32
    bf16 = mybir.dt.bfloat16
    fp8 = mybir.dt.float8e4
    DR = mybir.MatmulPerfMode.DoubleRow
    Exp = mybir.ActivationFunctionType.Exp
    Ident = mybir.ActivationFunctionType.Identity
    Ln = mybir.ActivationFunctionType.Ln

    nc = bass.Bass()

    # DRAM parameters (per-core shards, pre-tiled on host).
    # q/k projections run in fp8(e4m3) DoubleRow mode: inputs are absmax-
    # scaled to +-240 on host with GLOBAL scales (the SPMD program, incl.
    # the descale immediates, is shared by all cores); the Ident copy's
    # scale undoes the quantization scaling. The d-chunk PAIR layout
    # [128, 2, .] feeds the 256-deep virtual contraction of DoubleRow.
    # v (and everything downstream of the softmax) stays bf16: fp8 there
    # would put ~4% noise directly on the output, while on q/k it only
    # perturbs softmax logits by ~1%.
    xT = nc.declare_dram_parameter("xT", [ND, 128, S], bf16, isOutput=False)
    xqT = nc.declare_dram_parameter("xqT", [ND // 2, 128, 2, S], fp8, isOutput=False)
    wqfT = nc.declare_dram_parameter("wqfT", [ND // 2, 128, 2, GD], fp8, isOutput=False)
    wkfT = nc.declare_dram_parameter("wkfT", [ND // 2, 128, 2, GD], fp8, isOutput=False)
    wvT = nc.declare_dram_parameter("wvT", [128, ND * GD], bf16, isOutput=False)
    bq = nc.declare_dram_parameter("bq", [128, HPG], f32, isOutput=False)
    bk = nc.declare_dram_parameter("bk", [128, HPG], f32, isOutput=False)
    bvrow = nc.declare_dram_parameter("bvrow", [1, GD], bf16, isOutput=False)
    outwT = nc.declare_dram_parameter("outwT", [128, HPG * D], bf16, isOutput=False)
    keepT = nc.declare_dram_parameter("keepT", [NKC, 128, S], bf16, isOutput=False)
    out = nc.declare_dram_parameter("out", [S, D], bf16, isOutput=True)

    with tile.TileContext(nc) as tc:
        import contextlib

        with contextlib.ExitStack() as ctx:
            # --- pools ---
            # xT and keepT share one 16-slot rotation of [128, S] bf16 tiles.
            p_big = ctx.enter_context(tc.tile_pool(name="big2k", bufs=16))
            p_pers = ctx.enter_context(tc.tile_pool(name="pers", bufs=1))
            p_pm = ctx.enter_context(tc.tile_pool(name="pm", bufs=12))
            p_sm = ctx.enter_context(tc.tile_pool(name="small", bufs=2))
            pp_big = ctx.enter_context(tc.tile_pool(name="ppbig", bufs=2, space="PSUM"))
            pp_sm = ctx.enter_context(tc.tile_pool(name="ppsm", bufs=4, space="PSUM"))

            # --- constants + small inputs ---
            ones128 = p_pers.tile([128, 128], bf16, tag="ones128", name="ones128")
            nc.vector.memset(ones128, 1.0)

            # --- loads. Everything bandwidth-significant goes on the sync
            # HW DGE queue in strict consumption order (the HBM pipe is the
            # resource). DMA aggregate bandwidth ramps with queue depth, so
            # keep transfers outstanding — but split the first fp8 x tile
            # into quarters so the first matmuls' critical bytes land fast.
            wqf_sb = [
                p_pers.tile([128, 2, GD], fp8, tag=f"wqf{t}", name=f"wqf{t}")
                for t in range(ND // 2)
            ]
            xq_tiles = [
                p_pers.tile([128, 2, S], fp8, tag=f"xq{t}", name=f"xq{t}")
                for t in range(ND // 2)
            ]
            # first matmul's bytes first: wqf0, then xq0 in quarters
            nc.sync.dma_start(out=wqf_sb[0], in_=wqfT[0])
            for qu in range(4):
                nc.sync.dma_start(
                    out=xq_tiles[0][:, :, qu * 512:(qu + 1) * 512],
                    in_=xqT[0][:, :, qu * 512:(qu + 1) * 512],
                )
            for t in range(1, ND // 2):
                nc.sync.dma_start(out=wqf_sb[t], in_=wqfT[t])
                nc.sync.dma_start(out=xq_tiles[t], in_=xqT[t])
            wkf_sb = [
                p_pers.tile([128, 2, GD], fp8, tag=f"wkf{t}", name=f"wkf{t}")
                for t in range(ND // 2)
            ]
            for t in range(ND // 2):
                nc.sync.dma_start(out=wkf_sb[t], in_=wkfT[t])
            # bf16 x (for the v projection, which stays bf16)
            xt_tiles = []
            for d in range(ND):
                t = p_big.tile([128, S], bf16, tag="big2k", name="big2k")
                nc.sync.dma_start(out=t, in_=xT[d])
                xt_tiles.append(t)
            w_merged = {}
            wv_sb = p_pers.tile([128, ND * GD], bf16, tag="wv", name="wv")
            nc.sync.dma_start(out=wv_sb, in_=wvT[:, :])
            w_merged["v"] = wv_sb

            bq_sb = p_pers.tile([128, HPG], f32, tag="bq", name="bq_sb")
            nc.scalar.dma_start(out=bq_sb, in_=bq[:, :])
            bk_sb = p_pers.tile([128, HPG], f32, tag="bk", name="bk_sb")
            nc.scalar.dma_start(out=bk_sb, in_=bk[:, :])
            bv_sb = None
            if use_vbias:
                bv_sb = p_pers.tile([1, GD], bf16, tag="bv", name="bv_sb")
                nc.scalar.dma_start(out=bv_sb, in_=bvrow[:, :])

            def w_sl(name, d):
                return w_merged[name][:, d * GD:(d + 1) * GD]

            def xT_sl(d, lo, hi):
                return xt_tiles[d][:, lo:hi]

            keep_tiles = [None] * NKC
            for kc in range(8):
                t = p_big.tile([128, S], bf16, tag="big2k", name="big2k")
                nc.sync.dma_start(out=t, in_=keepT[kc])
                keep_tiles[kc] = t

            def keep_sl(kc, lo, hi):
                return keep_tiles[kc][:, lo:hi]

            outw_merged = p_pers.tile([128, HPG * D], bf16, tag="outw", name="outw")
            nc.sync.dma_start(out=outw_merged, in_=outwT[:, :])
            outw_sb = [outw_merged[:, h * D:(h + 1) * D] for h in range(HPG)]

            # --- q/k projections, fp8 DoubleRow (4 pair-passes over the
            # 1024-deep contraction instead of 8 bf16 passes). The first 8
            # units run pair-major across 8 concurrent PSUM accumulators
            # (4 pp_sm tiles + halves of 2 pp_big tiles): while xq tiles
            # stream in, the PE always has 8 matmuls ready per pair,
            # staying busy enough to hold the HAM clock warm. Remaining
            # units run unit-major; the Ident copy applies the fp8 descale
            # and bias. ---
            NT = ND // 2
            qT_sb = [p_pers.tile([128, S], bf16, tag=f"qT{h}", name=f"qT{h}") for h in range(HPG)]
            kT_sb = [p_pers.tile([128, S], bf16, tag=f"kT{h}", name=f"kT{h}") for h in range(HPG)]

            units = []  # (stream, head, quarter)
            for wf, dst, bias, dsc in (
                (wqf_sb, qT_sb, bq_sb, q_descale),
                (wkf_sb, kT_sb, bk_sb, k_descale),
            ):
                for h in range(HPG):
                    for qu in range(4):
                        units.append((wf, dst, bias, dsc, h, qu))

            # PE warm-up: the HAM clock gate needs ~3.4us of sustained
            # activity to unthrottle. Burn the initial DMA wait on dummy
            # matmuls over the ones tile so the first real matmuls run at
            # 2.4GHz instead of 1.2.
            warm_ps = pp_sm.tile([128, 128], f32, tag="ppsm", name="ppsm")
            for _ in range(WARMUP_MMS):
                nc.tensor.matmul(warm_ps, lhsT=ones128, rhs=ones128,
                                 start=True, stop=True)

            NWIDE = 8  # first units, pair-major over 8 accumulators
            wide = units[:NWIDE]
            pss = [
                pp_sm.tile([128, 512], f32, tag="ppsm", name="ppsm")
                for _ in range(4)
            ]
            big01 = [
                pp_big.tile([128, 1024], f32, tag="ppbig", name="ppbig")
                for _ in range(2)
            ]
            pss += [big01[0][:, 0:512], big01[0][:, 512:1024],
                    big01[1][:, 0:512], big01[1][:, 512:1024]]
            for t in range(NT):
                for (wf, dst, bias, dsc, h, qu), ps in zip(wide, pss):
                    nc.tensor.matmul(
                        ps,
                        lhsT=wf[t][:, :, h * 128:(h + 1) * 128],
                        rhs=xq_tiles[t][:, :, qu * 512:(qu + 1) * 512],
                        start=(t == 0),
                        stop=(t == NT - 1),
                        perf_mode=DR,
                    )
            for (wf, dst, bias, dsc, h, qu), ps in zip(wide, pss):
                nc.scalar.activation(
                    out=dst[h][:, qu * 512:(qu + 1) * 512],
                    in_=ps,
                    func=Ident,
                    scale=dsc,
                    bias=bias[:, h:h + 1],
                )

            for wf, dst, bias, dsc, h, qu in units[NWIDE:]:
                ps = pp_sm.tile([128, 512], f32, tag="ppsm", name="ppsm")
                for t in range(NT):
                    nc.tensor.matmul(
                        ps,
                        lhsT=wf[t][:, :, h * 128:(h + 1) * 128],
                        rhs=xq_tiles[t][:, :, qu * 512:(qu + 1) * 512],
                        start=(t == 0),
                        stop=(t == NT - 1),
                        perf_mode=DR,
                    )
                nc.scalar.activation(
                    out=dst[h][:, qu * 512:(qu + 1) * 512],
                    in_=ps,
                    func=Ident,
                    scale=dsc,
                    bias=bias[:, h:h + 1],
                )

            v_sb = [p_pers.tile([128, GD], bf16, tag=f"v{sc}", name=f"v{sc}") for sc in range(NSC)]
            for sc in range(NSC):
                ps = pp_sm.tile([128, GD], f32, tag="ppsm", name="ppsm")
                for d in range(ND):
                    nc.tensor.matmul(
                        ps,
                        lhsT=xT_sl(d, sc * 128, (sc + 1) * 128),
                        rhs=w_sl("v", d),
                        start=(d == 0),
                        stop=(d == ND - 1) and not use_vbias,
                    )
                if use_vbias:
                    # bias via K=1 ones row
                    nc.tensor.matmul(
                        ps,
                        lhsT=ones128[0:1, :],
                        rhs=bv_sb,
                        start=False,
                        stop=True,
                    )
                nc.vector.tensor_copy(v_sb[sc], ps)

            # --- second half of keepT (reuses xT slots once proj done).
            # Issued on sync: these dma_starts BLOCK the issuing engine on
            # the slot-free semaphore, so they must not ride the scalar
            # queue where they would stall the attention exps. ---
            for kc in range(8, NKC):
                t = p_big.tile([128, S], bf16, tag="big2k", name="big2k")
                nc.sync.dma_start(out=t, in_=keepT[kc])
                keep_tiles[kc] = t

            # --- attention. A single software pipeline runs ACROSS unit
            # boundaries: the deferred work queue (oT/dB consumes plus each
            # unit's softmax normalization) drains inside the NEXT unit's
            # kc loop, so the PE never waits for a unit's ln/exp/mul chain
            # at a boundary, and the scores-PSUM slots recycle while the
            # trailing exps of the previous unit are still in flight. ---
            oT_sb = [p_pers.tile([128, S], bf16, tag=f"oT{h}", name=f"oT{h}") for h in range(HPG)]
            DELAY = 4
            work = []  # deferred emission closures, popped in order

            for h in range(HPG):
                for half in range(2):
                    q0 = half * 1024
                    o_ps = [pp_sm.tile([128, 512], f32, tag="ppsm", name="ppsm") for _ in range(2)]
                    d_ps = [pp_sm.tile([128, 512], f32, tag="ppsm", name="ppsm") for _ in range(2)]

                    def consume(kc, pm, o_ps=o_ps, d_ps=d_ps, h=h):
                        for qq in range(2):
                            nc.tensor.matmul(
                                o_ps[qq],
                                lhsT=v_sb[kc][:, h * 128:(h + 1) * 128],
                                rhs=pm[:, qq * 512:(qq + 1) * 512],
                                start=(kc == 0),
                                stop=(kc == NKC - 1),
                            )
                        for qq in range(2):
                            nc.tensor.matmul(
                                d_ps[qq],
                                lhsT=ones128,
                                rhs=pm[:, qq * 512:(qq + 1) * 512],
                                start=(kc == 0),
                                stop=(kc == NKC - 1),
                            )

                    def norm(o_ps=o_ps, d_ps=d_ps, h=h, q0=q0):
                        for qq in range(2):
                            # 1/d via exp(-ln(d)) on ACT (same table set as
                            # the score exps; DVE reciprocal is ~6cyc/elem)
                            lnd = p_sm.tile([128, 512], f32, tag="lnd", name="lnd")
                            nc.scalar.activation(out=lnd, in_=d_ps[qq], func=Ln)
                            rdb = p_sm.tile([128, 512], f32, tag="rdb", name="rdb")
                            nc.scalar.activation(out=rdb, in_=lnd, func=Exp, scale=-1.0)
                            nc.vector.tensor_mul(
                                oT_sb[h][:, q0 + qq * 512:q0 + (qq + 1) * 512],
                                o_ps[qq],
                                rdb,
                            )

                    for kc in range(NKC):
                        sT = pp_big.tile([128, 1024], f32, tag="ppbig", name="ppbig")
                        for nn in range(2):
                            nc.tensor.matmul(
                                sT[:, nn * 512:(nn + 1) * 512],
                                lhsT=kT_sb[h][:, kc * 128:(kc + 1) * 128],
                                rhs=qT_sb[h][:, q0 + nn * 512:q0 + (nn + 1) * 512],
                                start=True,
                                stop=True,
                            )
                        p = p_pm.tile([128, 1024], bf16, tag="pm", name="pm")
                        nc.scalar.activation(out=p, in_=sT, func=Exp, scale=SCALE)
                        pm = p_pm.tile([128, 1024], bf16, tag="pm", name="pm")
                        nc.vector.tensor_mul(
                            pm, p, keep_sl(kc, q0, q0 + 1024)
                        )
                        work.append(lambda kc=kc, pm=pm, c=consume: c(kc, pm))
                        while len(work) > DELAY:
                            work.pop(0)()
                    work.append(norm)
            while work:
                work.pop(0)()

            # --- output projection (partial; host adds the two groups + bias).
            # Partial output in bf16: halves the DVE copy cost and the
            # output DMA traffic; host accumulates in f32. ---
            for sc in range(NSC):
                ps = pp_big.tile([128, 1024], f32, tag="ppbig", name="ppbig")
                for h in range(HPG):
                    for nn in range(2):
                        nc.tensor.matmul(
                            ps[:, nn * 512:(nn + 1) * 512],
                            lhsT=oT_sb[h][:, sc * 128:(sc + 1) * 128],
                            rhs=outw_sb[h][:, nn * 512:(nn + 1) * 512],
                            start=(h == 0),
                            stop=(h == HPG - 1),
                        )
                osb = p_sm.tile([128, 1024], bf16, tag="osb", name="osb")
                # copy on ACT (idle once the exps are done); DVE still owns
                # the last unit's mask/normalization work at this point
                nc.scalar.activation(out=osb, in_=ps, func=Ident)
                # out DMAs issue from sync: scalar must stay unblocked for
                # the Ident copies
                nc.sync.dma_start(out=out[sc * 128:(sc + 1) * 128, :], in_=osb)

    _split_waits(nc, mybir, maxw=1)
    return nc


def _prep_core_inputs(x, attn_mask, qkv_w, qkv_b, q_w, q_b, k_w, k_b, v_w, v_b,
                      out_w):
    """Host-side: fold projections, shard, pre-transpose/tile, cast."""
    f = np.float32
    x = np.asarray(x, f)
    qkv_w = np.asarray(qkv_w, f)
    qkv_b = np.asarray(qkv_b, f)
    Ws = {}
    bs = {}
    for i, (w, b) in enumerate(((q_w, q_b), (k_w, k_b), (v_w, v_b))):
        w = np.asarray(w, f)
        b = np.asarray(b, f)
        sl = slice(i * D, (i + 1) * D)
        Ws[i] = w @ qkv_w[sl]              # [D, D] effective
        bs[i] = b + w @ qkv_b[sl]          # [D]
    out_wT = np.ascontiguousarray(np.asarray(out_w, f).T)  # [D(hd), D(model)]

    keepT = (np.asarray(attn_mask).T == 0).astype(BF16)    # [k, q]
    keepT_t = np.ascontiguousarray(keepT).reshape(NKC, 128, S)

    xT_all = []
    for b_i in range(B):
        xb = np.ascontiguousarray(x[b_i].T.astype(BF16))   # [D, S]
        xT_all.append(xb.reshape(ND, 128, S))

    def dmaj(w):
        # [D, GD] -> [128, ND*GD] with d-chunk-major free layout
        return np.ascontiguousarray(
            w.reshape(ND, 128, GD).transpose(1, 0, 2).reshape(128, ND * GD)
        )

    def pairs(a, ncol):
        # [D, ncol] f32 (already quant-scaled) -> fp8 [ND//2, 128, 2, ncol]
        q = np.clip(a, -240.0, 240.0).astype(ml_dtypes.float8_e4m3fn)
        return np.ascontiguousarray(
            q.reshape(ND // 2, 2, 128, ncol).transpose(0, 2, 1, 3)
        )

    # global fp8 quantization scales (must be identical across cores: the
    # descale immediate is baked into the shared SPMD program)
    sx = 240.0 / max(float(np.abs(x).max()), 1e-30)
    swq = 240.0 / max(float(np.abs(Ws[0]).max()), 1e-30)
    swk = 240.0 / max(float(np.abs(Ws[1]).max()), 1e-30)

    maps = []
    for c in range(8):
        b_i = c % B
        g = c // B
        sl = slice(g * GD, (g + 1) * GD)
        xb_f32 = x[b_i].T  # [D, S]
        m = {
            "xT": xT_all[b_i],
            "xqT": pairs(xb_f32 * sx, S),
            "wqfT": pairs(Ws[0][sl].T * swq, GD),
            "wkfT": pairs(Ws[1][sl].T * swk, GD),
            "wvT": dmaj(Ws[2][sl].T.astype(BF16)),
            "bq": np.ascontiguousarray(bs[0][sl].reshape(HPG, 128).T.astype(f)),
            "bk": np.ascontiguousarray(bs[1][sl].reshape(HPG, 128).T.astype(f)),
            "bvrow": bs[2][sl].astype(BF16).reshape(1, GD),
            "outwT": np.ascontiguousarray(
                out_wT[sl].astype(BF16).reshape(HPG, 128, D)
                .transpose(1, 0, 2).reshape(128, HPG * D)
            ),
            "keepT": keepT_t,
        }
        maps.append(m)
    return maps, (sx, swq, swk)


def kernel(x, attn_mask, qkv_w, qkv_b, q_w, q_b, k_w, k_b, v_w, v_b,
           out_w, out_b, _trace=False):
    _install_ntff_hook_shim()
    from concourse.bass_utils import run_bass_kernel_spmd

    in_maps, (sx, swq, swk) = _prep_core_inputs(
        x, attn_mask, qkv_w, qkv_b, q_w, q_b, k_w, k_b, v_w, v_b, out_w
    )
    use_vbias = bool(np.any(np.asarray(in_maps[0]["bvrow"], np.float32) != 0))
    q_descale = 1.0 / (sx * swq)
    k_descale = 1.0 / (sx * swk)
    key = ("nc", use_vbias, q_descale, k_descale)
    if key not in _cached:
        _cached[key] = _build_program(
            use_vbias=use_vbias, q_descale=q_descale, k_descale=k_descale
        )
    nc = _cached[key]
    core_ids = list(range(8))
    try:
        res = run_bass_kernel_spmd(nc, in_maps, core_ids, trace=_trace)
    except Exception:
        # transient NRT device wedge: reset the PJRT client best-effort,
        # then retry once
        try:
            import jax

            jax.clear_backends()
        except Exception:
            pass
        res = run_bass_kernel_spmd(nc, in_maps, core_ids, trace=_trace)
    _cached["last_result"] = res

    out_b = np.asarray(out_b, np.float32)
    full = np.empty((B, S, D), np.float32)
    for b_i in range(B):
        full[b_i] = (
            res.results[b_i]["out"].astype(np.float32)
            + res.results[b_i + B]["out"].astype(np.float32)
            + out_b
        )
    return full

